# revision 57
# baseline (speedup 1.0000x reference)
"""Trainium2 Bass kernel for nn_AverageAttn (B=4, S=4096, D=H=1024, 8 cores).

out = igate * iQ + fgate * h, where
  avg  = causal cumulative average of iV along seq
  h    = relu(avg @ W1 + b1) @ W2 + b2
  ifg  = sigmoid(concat(iQ, h) @ Wg + bg);  igate, fgate = split(ifg)

Sharding: 8 cores = (batch b, seq half). Each core processes T=2048 tokens.

Design (baseline 250us -> ~203us; rel err 1.547e-2 < 2e-2 gate,
bit-deterministic, matches the host numerics simulator to 5 digits):
 - ALL matmuls fp8 e4m3 + DoubleRow (216ns per K=256 N=512 MM = 2x
   bf16; LDWEIGHTS overlaps).  PE work = 768 MMs ~= 166us; everything
   else is scheduled to hide behind it.
 - Fused average scan on DVE: s[t] = (w[t] + s[t-1]) * r[t] with
   host-prepared w[t] = v[t]/(t-1) (f16) and r[t] = (t-1)/t (f16,
   roundings chosen so the running product stays within ~1 ulp),
   initial = first-half mean (host, f32 exact).  One tensor_tensor_scan
   per chunk writes the causal cumulative average STRAIGHT into the fp8
   DoubleRow rhs layout -- no cumsum buffer, no 1/n multiply.
 - GpSimd does no tensor work: concurrent Pool TTs halve DVE
   throughput (measured).  Pool only triggers DMAs (incl. outputs).
 - Phases: PE warm-up (dummy DR MMs during the DMA wait, flips the HAM
   clock gate) -> igate Q-half staged to SBUF f16 (zq) -> fgate Q-half
   staged for chunks 0..1 (fz; just enough early PE work to cover the
   scan chain) -> FFN1 -> FFN2 -> per-chunk gates.  Gate chunks 2..7
   accumulate fgate Q+h in PSUM (48 MMs/chunk paces the drain engines
   with slack); staged chunks add fz on DVE.
 - Queue layout: Sync = qp8/wgtq/vw/qT/late weight slabs; Act = early
   weight JIT + all PSUM drains/sigmoids, with w1s[1..2] and wgtb[0..1]
   fetched at its idle head (emitted late they would queue behind the
   PE-paced drains and stall the PE ~3.5us each); Pool = constants +
   output DMAs (on Sync they head-of-line-block later weight fetches).
 - Last chunk block-outer with per-block drains + split output DMAs.
 - fp16 staging/output (same bytes as bf16, 8x less rounding).
"""

import numpy as np

B, S, D = 4, 4096, 1024
H = 1024
T = S // 2              # tokens per core
P = 128
ND = D // P             # 8 feature chunks
NG = 2 * D // P         # 16 gate chunks
NP = ND // 2            # 4 DoubleRow K-pairs
NBLK = 4                # 512-token matmul blocks
BT = T // NBLK          # 512


def _build_program():
    import contextlib
    import concourse.bass as bass  # noqa: F401
    import concourse.tile as tile
    from concourse import mybir, bacc

    f32 = mybir.dt.float32
    f16 = mybir.dt.float16
    bf16 = mybir.dt.bfloat16
    fp8 = mybir.dt.float8e4
    Relu = mybir.ActivationFunctionType.Relu
    Ident = mybir.ActivationFunctionType.Identity
    Sigm = mybir.ActivationFunctionType.Sigmoid
    DR = mybir.MatmulPerfMode.DoubleRow
    Add = mybir.AluOpType.add
    Mult = mybir.AluOpType.mult

    nc = bacc.Bacc("TRN2", target_bir_lowering=False)

    qTd = nc.dram_tensor("qTd", [ND, P, T], bf16, kind="ExternalInput")
    vwd = nc.dram_tensor("vwd", [ND, P, T], f16, kind="ExternalInput")
    carr = nc.dram_tensor("carr", [P, ND], f32, kind="ExternalInput")
    ratio = nc.dram_tensor("ratio", [P, T], f16, kind="ExternalInput")
    w1s = nc.dram_tensor("w1s", [ND, P, NP, 2, P], fp8, kind="ExternalInput")
    w2s = nc.dram_tensor("w2s", [ND, P, NP, 2, P], fp8, kind="ExternalInput")
    wgtq = nc.dram_tensor("wgtq", [ND, P, NP, 2, P], fp8,
                          kind="ExternalInput")
    wgtb = nc.dram_tensor("wgtb", [ND, P, NP, 2, P], fp8,
                          kind="ExternalInput")
    qp8d = nc.dram_tensor("qp8d", [NP, P, 2, T], fp8, kind="ExternalInput")
    wgb = nc.dram_tensor("wgb", [NG, P, NP, 2, P], fp8, kind="ExternalInput")
    bpk = nc.dram_tensor("bpk", [P, 2 * ND + NG], f32, kind="ExternalInput")
    o = nc.dram_tensor("o", [ND, P, T], f16, kind="ExternalOutput")

    with tile.TileContext(nc) as tc:
        ctx = contextlib.ExitStack()
        with ctx:
            NSTG = 2            # fgate chunks staged early (cover scans)
            cpool = ctx.enter_context(tc.tile_pool(name="consts", bufs=1))
            qpool = ctx.enter_context(tc.tile_pool(name="qq", bufs=ND))
            q8pool = ctx.enter_context(tc.tile_pool(name="q8", bufs=NP))
            s4k = ctx.enter_context(tc.tile_pool(name="s4k", bufs=NP))
            zqpool = ctx.enter_context(tc.tile_pool(name="zq", bufs=ND))
            fzpool = ctx.enter_context(tc.tile_pool(name="fz", bufs=NSTG))
            vpool = ctx.enter_context(tc.tile_pool(name="vstream", bufs=2))
            scanpool = ctx.enter_context(tc.tile_pool(name="scans", bufs=3))
            avgpool = ctx.enter_context(tc.tile_pool(name="avg", bufs=NP))
            hpool = ctx.enter_context(tc.tile_pool(name="hh", bufs=NP))
            wpool = ctx.enter_context(tc.tile_pool(name="w12", bufs=4))
            gwpool = ctx.enter_context(tc.tile_pool(name="gw", bufs=4))
            gbpool = ctx.enter_context(tc.tile_pool(name="gb", bufs=6))
            gatepool = ctx.enter_context(tc.tile_pool(name="gates", bufs=2))
            opool = ctx.enter_context(tc.tile_pool(name="outs", bufs=1))
            pspool = ctx.enter_context(
                tc.tile_pool(name="psmm", bufs=2, space="PSUM"))

            # ---- DMA wave --------------------------------------------------
            # Sync queue (fastest to start): carry, first igate slab, the
            # fp8 iQ pairs interleaved with the first v chunks (the scan
            # chain must start ASAP), then the rest of v (whose issues are
            # paced by scan slot-release), then qT (gates-phase only).
            gtiles = {}
            carT = cpool.tile([P, ND], f32, tag="carr")
            nc.sync.dma_start(carT[:], carr[:])
            t_ = gwpool.tile([P, NP, 2, P], fp8, tag="wgt", name="top")
            nc.sync.dma_start(t_[:], wgtq[0])
            gtiles[0] = t_
            qp8 = [q8pool.tile([P, 2, T], fp8, tag="qp8", name="qp8")
                   for _ in range(NP)]
            vts = [vpool.tile([P, T], f16, tag="v", name="vt")
                   for _ in range(ND)]
            for p in range(NP):
                nc.sync.dma_start(qp8[p][:, :, :T // 2], qp8d[p, :, :, :T // 2])
                nc.sync.dma_start(qp8[p][:, :, T // 2:], qp8d[p, :, :, T // 2:])
                if p >= 1:
                    nc.sync.dma_start(vts[p - 1][:], vwd[p - 1])
                if p == 1:
                    t_ = gwpool.tile([P, NP, 2, P], fp8, tag="wgt",
                                     name="top")
                    nc.sync.dma_start(t_[:], wgtq[1])
                    gtiles[1] = t_
            # first FFN1 slab rides ahead of the scan-paced v issues
            ftiles = {}
            w1tiles = {}
            w1t = wpool.tile([P, NP, 2, P], fp8, tag="w12", name="w1t")
            nc.sync.dma_start(w1t[:], w1s[0])
            w1tiles[0] = w1t
            for d in range(3, ND):
                nc.sync.dma_start(vts[d][:], vwd[d])
            qTc = [qpool.tile([P, T], bf16, tag="qT", name="qt")
                   for _ in range(ND)]
            for c in range(ND):
                nc.sync.dma_start(qTc[c][:], qTd[c])
            # ---- PE warm-up: dummy DR matmuls on memset tiles -------------
            # Runs during the initial DMA wait so the HAM clock-gate flips
            # to 8/8 before real matmuls start (cold MMs cost ~2x).
            wd = cpool.tile([P, 2, P], fp8, tag="wdum")
            xd = cpool.tile([P, 2, BT], fp8, tag="xdum")
            nc.gpsimd.memset(wd[:], 0.0)
            nc.gpsimd.memset(xd[:], 0.0)
            psd = pspool.tile([P, NBLK * BT], f32, tag="mm", name="psd")
            for r in range(11):
                nc.tensor.matmul(psd[:, :BT], wd[:], xd[:],
                                 start=(r == 0), stop=(r == 10),
                                 perf_mode=DR)
            scr0 = scanpool.tile([P, T], f16, tag="scan", name="scr0")
            nc.vector.tensor_copy(scr0[:, :BT], psd[:, :BT])

            # Pool queue: the ratio vector + biases (pool does no tensor
            # work at all -- concurrent GpSimd TTs halve DVE throughput).
            ratT = cpool.tile([P, T], f16, tag="ratio")
            bT = cpool.tile([P, 2 * ND + NG], f32, tag="bpk")
            nc.gpsimd.dma_start(ratT[:], ratio[:])
            nc.gpsimd.dma_start(bT[:], bpk[:])

            def qT(c):  # [P, T] view of iQ chunk c
                return qTc[c][:]

            # ---- fused average scan on DVE --------------------------------
            # s[t] = (w[t] + s[t-1]) * r[t] with host-prepared
            # w[t] = v[t]/(t-1), r[t] = (t-1)/t computes the causal
            # cumulative average directly (initial = first-half average),
            # written straight to the fp8 DoubleRow rhs layout.  No
            # separate cumsum buffer, no 1/n multiply.
            avg8 = [avgpool.tile([P, 2, T], fp8, tag="avg", name="avg8")
                    for _ in range(ND // 2)]
            for d in range(ND):
                nc.vector.tensor_tensor_scan(avg8[d // 2][:, d % 2, :],
                                             vts[d][:], ratT[:],
                                             carT[:, d:d + 1], Add, Mult)

            # ---- EARLY A: igate Q-half (fp8 DR), staged to SBUF fp16 ------
            zq = []
            for g in range(ND):
                gn = g + 2
                if gn < ND:
                    tn = gwpool.tile([P, NP, 2, P], fp8, tag="wgt",
                                     name="top")
                    nc.scalar.dma_start(tn[:], wgtq[gn])
                    gtiles[gn] = tn
                if g == 0:
                    # FFN1 slabs 1-2 + the early-B fgate slabs ride the
                    # idle head of the Act queue (their late emission
                    # would otherwise queue behind the PE-paced drains)
                    for jw in (1, 2):
                        wt_ = wpool.tile([P, NP, 2, P], fp8, tag="w12",
                                         name="w1t")
                        nc.scalar.dma_start(wt_[:], w1s[jw])
                        w1tiles[jw] = wt_
                    for gb_ in range(NSTG):
                        tb_ = gbpool.tile([P, NP, 2, P], fp8, tag="wgb",
                                          name="topf")
                        nc.scalar.dma_start(tb_[:], wgtb[gb_])
                        ftiles[gb_] = tb_
                top = gtiles[g]
                ps = pspool.tile([P, NBLK * BT], f32, tag="mm", name="ps")
                for p in range(NP):
                    for blk in range(NBLK):
                        nc.tensor.matmul(
                            ps[:, blk * BT:(blk + 1) * BT], top[:, p, :, :],
                            qp8[p][:, :, blk * BT:(blk + 1) * BT],
                            start=(p == 0), stop=(p == NP - 1), perf_mode=DR)
                zt = zqpool.tile([P, T], f16, tag="zq", name="zq")
                nc.scalar.activation(zt[:], ps[:], Ident)
                zq.append(zt)

            # ---- EARLY B: fgate Q-half for chunks 0..NSTG-1, staged -------
            # (just enough extra early PE work to cover the scan chain;
            # the rest of the fgate Q-halves run in the gates phase where
            # the thicker PE load paces the drain engines.)
            fz = []
            for g in range(NSTG):
                top = ftiles[g]
                ps = pspool.tile([P, NBLK * BT], f32, tag="mm", name="ps")
                for p in range(NP):
                    for blk in range(NBLK):
                        nc.tensor.matmul(
                            ps[:, blk * BT:(blk + 1) * BT], top[:, p, :, :],
                            qp8[p][:, :, blk * BT:(blk + 1) * BT],
                            start=(p == 0), stop=(p == NP - 1), perf_mode=DR)
                zt = fzpool.tile([P, T], f16, tag="fz", name="fz")
                nc.scalar.activation(zt[:], ps[:], Ident)
                fz.append(zt)

            # ---- FFN1: h1 = relu(avg @ W1 + b1), fp8 DoubleRow ------------
            # (w1s[0] fetched in the DMA wave; w1s[1..2] on early Act)
            h1 = [s4k.tile([P, 2, T], fp8, tag="s4k", name="h1")
                  for _ in range(NP)]
            for j in range(ND):
                jn = j + 3
                if jn < ND:
                    wn = wpool.tile([P, NP, 2, P], fp8, tag="w12", name="w1t")
                    nc.sync.dma_start(wn[:], w1s[jn])
                    w1tiles[jn] = wn
                w1t = w1tiles[j]
                ps = pspool.tile([P, NBLK * BT], f32, tag="mm", name="ps")
                for p in range(NP):
                    for blk in range(NBLK):
                        nc.tensor.matmul(
                            ps[:, blk * BT:(blk + 1) * BT], w1t[:, p, :, :],
                            avg8[p][:, :, blk * BT:(blk + 1) * BT],
                            start=(p == 0), stop=(p == NP - 1),
                            perf_mode=DR)
                nc.scalar.activation(h1[j // 2][:, j % 2, :], ps[:], Relu,
                                     bias=bT[:, j:j + 1])

            # ---- FFN2: h = h1 @ W2 + b2, fp8 DoubleRow --------------------
            w2tiles = {}
            for j in (0, 1):
                w2t = wpool.tile([P, NP, 2, P], fp8, tag="w12", name="w2t")
                nc.sync.dma_start(w2t[:], w2s[j])
                w2tiles[j] = w2t
            hh = [hpool.tile([P, 2, T], fp8, tag="hh", name="hh")
                  for _ in range(NP)]
            for j in range(ND):
                jn = j + 2
                if jn < ND:
                    wn = wpool.tile([P, NP, 2, P], fp8, tag="w12", name="w2t")
                    nc.sync.dma_start(wn[:], w2s[jn])
                    w2tiles[jn] = wn
                w2t = w2tiles[j]
                ps = pspool.tile([P, NBLK * BT], f32, tag="mm", name="ps")
                for p in range(NP):
                    for blk in range(NBLK):
                        nc.tensor.matmul(
                            ps[:, blk * BT:(blk + 1) * BT], w2t[:, p, :, :],
                            h1[p][:, :, blk * BT:(blk + 1) * BT],
                            start=(p == 0), stop=(p == NP - 1),
                            perf_mode=DR)
                nc.scalar.activation(hh[j // 2][:, j % 2, :], ps[:], Ident,
                                     bias=bT[:, ND + j:ND + j + 1])

            # ---- gates + output -------------------------------------------
            # igate: h-half psum -> Act drain f16 -> DVE add (+zq) -> sigm.
            # fgate: h-half psum -> DVE add (psum + fz) -> Act sigmoid.
            btiles = {}

            def fetch(kind, idx):
                if kind == 'gq':
                    src, off = wgtb, idx
                else:
                    src, off = wgb, (idx + ND if kind == 'gf' else idx)
                t_ = gbpool.tile([P, NP, 2, P], fp8, tag="wgb", name=kind)
                nc.sync.dma_start(t_[:], src[off])
                btiles[(kind, idx)] = t_

            fetch('gi', 0)
            fetch('gf', 0)
            fetch('gi', 1)
            fetch('gq', NSTG)

            for gp in range(ND):
                last = gp == ND - 1
                staged = gp < NSTG
                if gp + 1 < ND:
                    if gp + 1 > NSTG:
                        fetch('gq', gp + 1)
                    fetch('gf', gp + 1)
                    if gp + 2 < ND:
                        fetch('gi', gp + 2)
                nb = NBLK if last else 1
                bw = T // nb
                wi = btiles[('gi', gp)]
                wf = btiles[('gf', gp)]

                # igate h-half
                ps = pspool.tile([P, NBLK * BT], f32, tag="mm", name="ps")
                for x in range(nb):
                    nbl = bw // BT
                    for p in range(NP):
                        for b2 in range(nbl):
                            s2 = slice(x * bw + b2 * BT,
                                       x * bw + (b2 + 1) * BT)
                            nc.tensor.matmul(
                                ps[:, s2], wi[:, p, :, :], hh[p][:, :, s2],
                                start=(p == 0), stop=(p == NP - 1),
                                perf_mode=DR)
                ig = gatepool.tile([P, T], f16, tag="gate", name="ig")
                zi2 = scanpool.tile([P, T], f16, tag="scan", name="zi2")
                for x in range(nb):
                    sl = slice(x * bw, (x + 1) * bw)
                    nc.scalar.activation(zi2[:, sl], ps[:, sl], Ident)
                    nc.vector.tensor_add(zi2[:, sl], zi2[:, sl],
                                         zq[gp][:, sl])
                    nc.scalar.activation(ig[:, sl], zi2[:, sl], Sigm,
                                         bias=bT[:, 2 * ND + gp:
                                                 2 * ND + gp + 1])

                # fgate: staged chunks add the pre-staged Q-half (fz);
                # unstaged chunks accumulate Q-half + h-half in psum and
                # sigmoid straight from PSUM.
                g = gp + ND
                ps2 = pspool.tile([P, NBLK * BT], f32, tag="mm", name="ps2")
                for x in range(nb):
                    nbl = bw // BT
                    if not staged:
                        wq = btiles[('gq', gp)]
                        for p in range(NP):
                            for b2 in range(nbl):
                                s2 = slice(x * bw + b2 * BT,
                                           x * bw + (b2 + 1) * BT)
                                nc.tensor.matmul(
                                    ps2[:, s2], wq[:, p, :, :],
                                    qp8[p][:, :, s2],
                                    start=(p == 0), stop=False,
                                    perf_mode=DR)
                    for p in range(NP):
                        for b2 in range(nbl):
                            s2 = slice(x * bw + b2 * BT,
                                       x * bw + (b2 + 1) * BT)
                            nc.tensor.matmul(
                                ps2[:, s2], wf[:, p, :, :], hh[p][:, :, s2],
                                start=(staged and p == 0),
                                stop=(p == NP - 1), perf_mode=DR)
                fg = gatepool.tile([P, T], f16, tag="gate", name="fg")
                tmp = scanpool.tile([P, T], f16, tag="scan", name="tmp")
                ob = opool.tile([P, T], f16, tag="ob", name="ob")
                for x in range(nb):
                    sl = slice(x * bw, (x + 1) * bw)
                    if staged:
                        nc.vector.tensor_add(fz[gp][:, sl], ps2[:, sl],
                                             fz[gp][:, sl])
                        nc.scalar.activation(fg[:, sl], fz[gp][:, sl], Sigm,
                                             bias=bT[:, 2 * ND + g:
                                                     2 * ND + g + 1])
                    else:
                        nc.scalar.activation(fg[:, sl], ps2[:, sl], Sigm,
                                             bias=bT[:, 2 * ND + g:
                                                     2 * ND + g + 1])
                    nc.vector.tensor_mul(tmp[:, sl], ig[:, sl],
                                         qT(gp)[:, sl])
                    nc.vector.tensor_mul(ob[:, sl], fg[:, sl],
                                         hh[gp // 2][:, gp % 2, sl])
                    nc.vector.tensor_add(ob[:, sl], ob[:, sl], tmp[:, sl])
                    dmae = nc.scalar if (last and x >= 2) else nc.gpsimd
                    dmae.dma_start(o[gp, :, sl], ob[:, sl])

    nc.finalize()
    return nc


_CACHED = {}
_last_result = None


def kernel(iQ, iV, W1, b1, W2, b2, Wg, bg):
    import sys
    if '/opt/trn_rl_repo' not in sys.path:
        sys.path.insert(0, '/opt/trn_rl_repo')
    import ml_dtypes
    from concourse.bass_utils import run_bass_kernel_spmd

    BF = ml_dtypes.bfloat16
    F16 = np.float16
    F8 = ml_dtypes.float8_e4m3

    iQ = np.asarray(iQ, np.float32)
    iV = np.asarray(iV, np.float32)
    W1 = np.asarray(W1, np.float32)
    b1 = np.asarray(b1, np.float32)
    W2 = np.asarray(W2, np.float32)
    b2 = np.asarray(b2, np.float32)
    Wg = np.asarray(Wg, np.float32)
    bg = np.asarray(bg, np.float32)

    if 'nc' not in _CACHED:
        _CACHED['nc'] = _build_program()
    nc = _CACHED['nc']

    # weight slabs, lhsT layouts (see _build_program dram shapes)
    def dr_slab(W, n_out):
        # [j, k, p, i, m] with K index (p*2+i)*128+k
        return np.ascontiguousarray(
            W.reshape(NP, 2, P, n_out, P).transpose(3, 2, 0, 1, 4)).astype(F8)

    w1s = dr_slab(W1, ND)
    w2s = dr_slab(W2, ND)
    wgtq = dr_slab(Wg[:D, :D], ND)
    wgtb = dr_slab(Wg[:D, D:], ND)
    wgb = dr_slab(Wg[D:], NG)
    bpk = np.ascontiguousarray(np.concatenate([
        b1.reshape(ND, P).T, b2.reshape(ND, P).T, bg.reshape(NG, P).T],
        axis=1))

    def t_orient(x, dt):  # [T, D] f32 -> [ND, P, T]
        return np.ascontiguousarray(x.T.reshape(ND, P, T)).astype(dt)

    def comp_ratio(n0):
        # f16 ratios r[t] ~ (n-1)/n chosen so the running product tracks
        # the ideal product to ~1 ulp (errors would otherwise compound
        # as a random walk across the 2048-step scan).
        r = np.empty(T, np.float32)
        perr = 1.0
        for i in range(T):
            n = n0 + i
            ideal = (n - 1) / n if n > 1 else 1.0
            c = np.float32(ideal / perr)
            cand = (np.nextafter(np.float16(c), np.float16(0)),
                    np.float16(c),
                    np.nextafter(np.float16(c), np.float16(2)))
            best = min((float(x) for x in cand),
                       key=lambda rv: abs(perr * rv / ideal - 1))
            r[i] = best
            perr = perr * best / ideal
        return r

    rats = {h: comp_ratio(h * T + 1) for h in (0, 1)}

    in_maps = []
    for core in range(8):
        b, half = core // 2, core % 2
        off = half * T
        ns = np.arange(off + 1, off + T + 1, dtype=np.float64)
        div = np.maximum(ns - 1, 1.0)
        rat = np.ascontiguousarray(
            np.broadcast_to(rats[half], (P, T))).astype(F16)
        if half == 1:
            car = np.ascontiguousarray(
                iV[b, :T].mean(axis=0, dtype=np.float64)
                .astype(np.float32).reshape(ND, P).T)
        else:
            car = np.zeros((P, ND), np.float32)
        m = {
            "qTd": t_orient(iQ[b, off:off + T], BF),
            "vwd": t_orient(
                (iV[b, off:off + T] / div[:, None]).astype(np.float32), F16),
            "carr": car,
            "ratio": rat,
            "w1s": w1s, "w2s": w2s, "wgb": wgb, "wgtb": wgtb,
            "wgtq": wgtq,
            "qp8d": np.ascontiguousarray(
                iQ[b, off:off + T].T.reshape(NP, 2, P, T)
                .transpose(0, 2, 1, 3)).astype(F8),
            "bpk": bpk,
        }
        in_maps.append(m)

    res = run_bass_kernel_spmd(nc, in_maps, core_ids=list(range(8)))
    global _last_result
    _last_result = res

    out = np.empty((B, S, D), np.float32)
    for core in range(8):
        b, half = core // 2, core % 2
        ot = np.asarray(res.results[core]["o"], dtype=np.float32)
        out[b, half * T:(half + 1) * T] = \
            ot.transpose(2, 0, 1).reshape(T, D)
    return out


# revision 58
# speedup vs baseline: 1.0227x; 1.0227x over previous
"""Trainium2 Bass kernel for nn_AverageAttn (B=4, S=4096, D=H=1024, 8 cores).

out = igate * iQ + fgate * h, where
  avg  = causal cumulative average of iV along seq
  h    = relu(avg @ W1 + b1) @ W2 + b2
  ifg  = sigmoid(concat(iQ, h) @ Wg + bg);  igate, fgate = split(ifg)

Sharding: 8 cores = (batch b, seq half). Each core processes T=2048 tokens.

Design (baseline 250us -> ~203us; rel err 1.547e-2 < 2e-2 gate,
bit-deterministic, matches the host numerics simulator to 5 digits):
 - ALL matmuls fp8 e4m3 + DoubleRow (216ns per K=256 N=512 MM = 2x
   bf16; LDWEIGHTS overlaps).  PE work = 768 MMs ~= 166us; everything
   else is scheduled to hide behind it.
 - Fused average scan on DVE: s[t] = (w[t] + s[t-1]) * r[t] with
   host-prepared w[t] = v[t]/(t-1) (f16) and r[t] = (t-1)/t (f16,
   roundings chosen so the running product stays within ~1 ulp),
   initial = first-half mean (host, f32 exact).  One tensor_tensor_scan
   per chunk writes the causal cumulative average STRAIGHT into the fp8
   DoubleRow rhs layout -- no cumsum buffer, no 1/n multiply.
 - GpSimd does no tensor work: concurrent Pool TTs halve DVE
   throughput (measured).  Pool only triggers DMAs (incl. outputs).
 - Phases: PE warm-up (dummy DR MMs during the DMA wait, flips the HAM
   clock gate) -> igate Q-half staged to SBUF f16 (zq) -> fgate Q-half
   staged for chunks 0..1 (fz; just enough early PE work to cover the
   scan chain) -> FFN1 -> FFN2 -> per-chunk gates.  Gate chunks 2..7
   accumulate fgate Q+h in PSUM (48 MMs/chunk paces the drain engines
   with slack); staged chunks add fz on DVE.
 - Queue layout: Sync = qp8/wgtq/vw/qT/late weight slabs; Act = early
   weight JIT + all PSUM drains/sigmoids, with w1s[1..2] and wgtb[0..1]
   fetched at its idle head (emitted late they would queue behind the
   PE-paced drains and stall the PE ~3.5us each); Pool = constants +
   output DMAs (on Sync they head-of-line-block later weight fetches).
 - Last chunk block-outer with per-block drains + split output DMAs.
 - fp16 staging/output (same bytes as bf16, 8x less rounding).
"""

import numpy as np

B, S, D = 4, 4096, 1024
H = 1024
T = S // 2              # tokens per core
P = 128
ND = D // P             # 8 feature chunks
NG = 2 * D // P         # 16 gate chunks
NP = ND // 2            # 4 DoubleRow K-pairs
NBLK = 4                # 512-token matmul blocks
BT = T // NBLK          # 512


def _build_program():
    import contextlib
    import concourse.bass as bass  # noqa: F401
    import concourse.tile as tile
    from concourse import mybir, bacc

    f32 = mybir.dt.float32
    f16 = mybir.dt.float16
    bf16 = mybir.dt.bfloat16
    fp8 = mybir.dt.float8e4
    Relu = mybir.ActivationFunctionType.Relu
    Ident = mybir.ActivationFunctionType.Identity
    Sigm = mybir.ActivationFunctionType.Sigmoid
    DR = mybir.MatmulPerfMode.DoubleRow
    Add = mybir.AluOpType.add
    Mult = mybir.AluOpType.mult

    nc = bacc.Bacc("TRN2", target_bir_lowering=False)

    qTd = nc.dram_tensor("qTd", [ND, P, T], bf16, kind="ExternalInput")
    vwd = nc.dram_tensor("vwd", [ND, P, T], f16, kind="ExternalInput")
    carr = nc.dram_tensor("carr", [P, ND], f32, kind="ExternalInput")
    ratio = nc.dram_tensor("ratio", [P, T], f16, kind="ExternalInput")
    w1s = nc.dram_tensor("w1s", [ND, P, NP, 2, P], fp8, kind="ExternalInput")
    w2s = nc.dram_tensor("w2s", [ND, P, NP, 2, P], fp8, kind="ExternalInput")
    wgtq = nc.dram_tensor("wgtq", [ND, P, NP, 2, P], fp8,
                          kind="ExternalInput")
    wgtb = nc.dram_tensor("wgtb", [ND, P, NP, 2, P], fp8,
                          kind="ExternalInput")
    qp8d = nc.dram_tensor("qp8d", [NP, P, 2, T], fp8, kind="ExternalInput")
    wgb = nc.dram_tensor("wgb", [NG, P, NP, 2, P], fp8, kind="ExternalInput")
    bpk = nc.dram_tensor("bpk", [P, 2 * ND + NG], f32, kind="ExternalInput")
    o = nc.dram_tensor("o", [ND, P, T], f16, kind="ExternalOutput")

    with tile.TileContext(nc) as tc:
        ctx = contextlib.ExitStack()
        with ctx:
            NSTG = 2            # fgate chunks staged early (cover scans)
            cpool = ctx.enter_context(tc.tile_pool(name="consts", bufs=1))
            qpool = ctx.enter_context(tc.tile_pool(name="qq", bufs=ND))
            q8pool = ctx.enter_context(tc.tile_pool(name="q8", bufs=NP))
            s4k = ctx.enter_context(tc.tile_pool(name="s4k", bufs=NP))
            zqpool = ctx.enter_context(tc.tile_pool(name="zq", bufs=ND))
            fzpool = ctx.enter_context(tc.tile_pool(name="fz", bufs=NSTG))
            vpool = ctx.enter_context(tc.tile_pool(name="vstream", bufs=2))
            scanpool = ctx.enter_context(tc.tile_pool(name="scans", bufs=2))
            avgpool = ctx.enter_context(tc.tile_pool(name="avg", bufs=NP))
            hpool = ctx.enter_context(tc.tile_pool(name="hh", bufs=NP))
            wpool = ctx.enter_context(tc.tile_pool(name="w12", bufs=4))
            gwpool = ctx.enter_context(tc.tile_pool(name="gw", bufs=4))
            gbpool = ctx.enter_context(tc.tile_pool(name="gb", bufs=6))
            gatepool = ctx.enter_context(tc.tile_pool(name="gates", bufs=2))
            opool = ctx.enter_context(tc.tile_pool(name="outs", bufs=2))
            pspool = ctx.enter_context(
                tc.tile_pool(name="psmm", bufs=2, space="PSUM"))

            # ---- DMA wave --------------------------------------------------
            # Sync queue (fastest to start): carry, first igate slab, the
            # fp8 iQ pairs interleaved with the first v chunks (the scan
            # chain must start ASAP), then the rest of v (whose issues are
            # paced by scan slot-release), then qT (gates-phase only).
            gtiles = {}
            carT = cpool.tile([P, ND], f32, tag="carr")
            nc.sync.dma_start(carT[:], carr[:])
            t_ = gwpool.tile([P, NP, 2, P], fp8, tag="wgt", name="top")
            nc.sync.dma_start(t_[:], wgtq[0])
            gtiles[0] = t_
            qp8 = [q8pool.tile([P, 2, T], fp8, tag="qp8", name="qp8")
                   for _ in range(NP)]
            vts = [vpool.tile([P, T], f16, tag="v", name="vt")
                   for _ in range(ND)]
            for p in range(NP):
                nc.sync.dma_start(qp8[p][:, :, :T // 2], qp8d[p, :, :, :T // 2])
                nc.sync.dma_start(qp8[p][:, :, T // 2:], qp8d[p, :, :, T // 2:])
                if p >= 1:
                    nc.sync.dma_start(vts[p - 1][:], vwd[p - 1])
                if p == 1:
                    t_ = gwpool.tile([P, NP, 2, P], fp8, tag="wgt",
                                     name="top")
                    nc.sync.dma_start(t_[:], wgtq[1])
                    gtiles[1] = t_
            # first FFN1 slab rides ahead of the scan-paced v issues
            ftiles = {}
            w1tiles = {}
            w1t = wpool.tile([P, NP, 2, P], fp8, tag="w12", name="w1t")
            nc.sync.dma_start(w1t[:], w1s[0])
            w1tiles[0] = w1t
            for d in range(3, ND):
                nc.sync.dma_start(vts[d][:], vwd[d])
            qTc = [qpool.tile([P, T], bf16, tag="qT", name="qt")
                   for _ in range(ND)]
            for c in range(ND):
                nc.sync.dma_start(qTc[c][:], qTd[c])
            # ---- PE warm-up: dummy DR matmuls on memset tiles -------------
            # Runs during the initial DMA wait so the HAM clock-gate flips
            # to 8/8 before real matmuls start (cold MMs cost ~2x).
            wd = cpool.tile([P, 2, P], fp8, tag="wdum")
            xd = cpool.tile([P, 2, BT], fp8, tag="xdum")
            nc.gpsimd.memset(wd[:], 0.0)
            nc.gpsimd.memset(xd[:], 0.0)
            psd = pspool.tile([P, NBLK * BT], f32, tag="mm", name="psd")
            for r in range(11):
                nc.tensor.matmul(psd[:, :BT], wd[:], xd[:],
                                 start=(r == 0), stop=(r == 10),
                                 perf_mode=DR)
            scr0 = scanpool.tile([P, T], f16, tag="scan", name="scr0")
            nc.vector.tensor_copy(scr0[:, :BT], psd[:, :BT])

            # Pool queue: the ratio vector + biases (pool does no tensor
            # work at all -- concurrent GpSimd TTs halve DVE throughput).
            ratT = cpool.tile([P, T], f16, tag="ratio")
            bT = cpool.tile([P, 2 * ND + NG], f32, tag="bpk")
            nc.gpsimd.dma_start(ratT[:], ratio[:])
            nc.gpsimd.dma_start(bT[:], bpk[:])

            def qT(c):  # [P, T] view of iQ chunk c
                return qTc[c][:]

            # ---- fused average scan on DVE --------------------------------
            # s[t] = (w[t] + s[t-1]) * r[t] with host-prepared
            # w[t] = v[t]/(t-1), r[t] = (t-1)/t computes the causal
            # cumulative average directly (initial = first-half average),
            # written straight to the fp8 DoubleRow rhs layout.  No
            # separate cumsum buffer, no 1/n multiply.
            avg8 = [avgpool.tile([P, 2, T], fp8, tag="avg", name="avg8")
                    for _ in range(ND // 2)]
            for d in range(ND):
                nc.vector.tensor_tensor_scan(avg8[d // 2][:, d % 2, :],
                                             vts[d][:], ratT[:],
                                             carT[:, d:d + 1], Add, Mult)

            # ---- EARLY A: igate Q-half (fp8 DR), staged to SBUF fp16 ------
            zq = []
            for g in range(ND):
                gn = g + 2
                if gn < ND:
                    tn = gwpool.tile([P, NP, 2, P], fp8, tag="wgt",
                                     name="top")
                    nc.scalar.dma_start(tn[:], wgtq[gn])
                    gtiles[gn] = tn
                if g == 0:
                    # FFN1 slabs 1-2 + the early-B fgate slabs ride the
                    # idle head of the Act queue (their late emission
                    # would otherwise queue behind the PE-paced drains)
                    for jw in (1, 2):
                        wt_ = wpool.tile([P, NP, 2, P], fp8, tag="w12",
                                         name="w1t")
                        nc.scalar.dma_start(wt_[:], w1s[jw])
                        w1tiles[jw] = wt_
                    for gb_ in range(NSTG):
                        tb_ = gbpool.tile([P, NP, 2, P], fp8, tag="wgb",
                                          name="topf")
                        nc.scalar.dma_start(tb_[:], wgtb[gb_])
                        ftiles[gb_] = tb_
                top = gtiles[g]
                ps = pspool.tile([P, NBLK * BT], f32, tag="mm", name="ps")
                for p in range(NP):
                    for blk in range(NBLK):
                        nc.tensor.matmul(
                            ps[:, blk * BT:(blk + 1) * BT], top[:, p, :, :],
                            qp8[p][:, :, blk * BT:(blk + 1) * BT],
                            start=(p == 0), stop=(p == NP - 1), perf_mode=DR)
                zt = zqpool.tile([P, T], f16, tag="zq", name="zq")
                nc.scalar.activation(zt[:], ps[:], Ident)
                zq.append(zt)

            # ---- EARLY B: fgate Q-half for chunks 0..NSTG-1, staged -------
            # (just enough extra early PE work to cover the scan chain;
            # the rest of the fgate Q-halves run in the gates phase where
            # the thicker PE load paces the drain engines.)
            fz = []
            for g in range(NSTG):
                top = ftiles[g]
                ps = pspool.tile([P, NBLK * BT], f32, tag="mm", name="ps")
                for p in range(NP):
                    for blk in range(NBLK):
                        nc.tensor.matmul(
                            ps[:, blk * BT:(blk + 1) * BT], top[:, p, :, :],
                            qp8[p][:, :, blk * BT:(blk + 1) * BT],
                            start=(p == 0), stop=(p == NP - 1), perf_mode=DR)
                zt = fzpool.tile([P, T], f16, tag="fz", name="fz")
                nc.scalar.activation(zt[:], ps[:], Ident)
                fz.append(zt)

            # ---- FFN1: h1 = relu(avg @ W1 + b1), fp8 DoubleRow ------------
            # (w1s[0] fetched in the DMA wave; w1s[1..2] on early Act)
            h1 = [s4k.tile([P, 2, T], fp8, tag="s4k", name="h1")
                  for _ in range(NP)]
            for j in range(ND):
                jn = j + 3
                if jn < ND:
                    wn = wpool.tile([P, NP, 2, P], fp8, tag="w12", name="w1t")
                    nc.sync.dma_start(wn[:], w1s[jn])
                    w1tiles[jn] = wn
                w1t = w1tiles[j]
                ps = pspool.tile([P, NBLK * BT], f32, tag="mm", name="ps")
                for p in range(NP):
                    for blk in range(NBLK):
                        nc.tensor.matmul(
                            ps[:, blk * BT:(blk + 1) * BT], w1t[:, p, :, :],
                            avg8[p][:, :, blk * BT:(blk + 1) * BT],
                            start=(p == 0), stop=(p == NP - 1),
                            perf_mode=DR)
                nc.scalar.activation(h1[j // 2][:, j % 2, :], ps[:], Relu,
                                     bias=bT[:, j:j + 1])

            # ---- FFN2: h = h1 @ W2 + b2, fp8 DoubleRow --------------------
            w2tiles = {}
            for j in (0, 1):
                w2t = wpool.tile([P, NP, 2, P], fp8, tag="w12", name="w2t")
                nc.sync.dma_start(w2t[:], w2s[j])
                w2tiles[j] = w2t
            hh = [hpool.tile([P, 2, T], fp8, tag="hh", name="hh")
                  for _ in range(NP)]
            for j in range(ND):
                jn = j + 2
                if jn < ND:
                    wn = wpool.tile([P, NP, 2, P], fp8, tag="w12", name="w2t")
                    nc.sync.dma_start(wn[:], w2s[jn])
                    w2tiles[jn] = wn
                w2t = w2tiles[j]
                ps = pspool.tile([P, NBLK * BT], f32, tag="mm", name="ps")
                for p in range(NP):
                    for blk in range(NBLK):
                        nc.tensor.matmul(
                            ps[:, blk * BT:(blk + 1) * BT], w2t[:, p, :, :],
                            h1[p][:, :, blk * BT:(blk + 1) * BT],
                            start=(p == 0), stop=(p == NP - 1),
                            perf_mode=DR)
                nc.scalar.activation(hh[j // 2][:, j % 2, :], ps[:], Ident,
                                     bias=bT[:, ND + j:ND + j + 1])

            # ---- gates + output -------------------------------------------
            # igate: h-half psum -> Act drain f16 -> DVE add (+zq) -> sigm.
            # fgate: h-half psum -> DVE add (psum + fz) -> Act sigmoid.
            btiles = {}

            def fetch(kind, idx):
                if kind == 'gq':
                    src, off = wgtb, idx
                else:
                    src, off = wgb, (idx + ND if kind == 'gf' else idx)
                t_ = gbpool.tile([P, NP, 2, P], fp8, tag="wgb", name=kind)
                nc.sync.dma_start(t_[:], src[off])
                btiles[(kind, idx)] = t_

            fetch('gi', 0)
            fetch('gf', 0)
            fetch('gi', 1)
            fetch('gq', NSTG)

            for gp in range(ND):
                last = gp == ND - 1
                staged = gp < NSTG
                if gp + 1 < ND:
                    if gp + 1 > NSTG:
                        fetch('gq', gp + 1)
                    fetch('gf', gp + 1)
                    if gp + 2 < ND:
                        fetch('gi', gp + 2)
                nb = NBLK if last else 1
                bw = T // nb
                wi = btiles[('gi', gp)]
                wf = btiles[('gf', gp)]

                # igate h-half
                ps = pspool.tile([P, NBLK * BT], f32, tag="mm", name="ps")
                for x in range(nb):
                    nbl = bw // BT
                    for p in range(NP):
                        for b2 in range(nbl):
                            s2 = slice(x * bw + b2 * BT,
                                       x * bw + (b2 + 1) * BT)
                            nc.tensor.matmul(
                                ps[:, s2], wi[:, p, :, :], hh[p][:, :, s2],
                                start=(p == 0), stop=(p == NP - 1),
                                perf_mode=DR)
                ig = gatepool.tile([P, T], f16, tag="gate", name="ig")
                zi2 = scanpool.tile([P, T], f16, tag="scan", name="zi2")
                for x in range(nb):
                    sl = slice(x * bw, (x + 1) * bw)
                    nc.scalar.activation(zi2[:, sl], ps[:, sl], Ident)
                    nc.vector.tensor_add(zi2[:, sl], zi2[:, sl],
                                         zq[gp][:, sl])
                    nc.scalar.activation(ig[:, sl], zi2[:, sl], Sigm,
                                         bias=bT[:, 2 * ND + gp:
                                                 2 * ND + gp + 1])

                # fgate: staged chunks add the pre-staged Q-half (fz);
                # unstaged chunks accumulate Q-half + h-half in psum and
                # sigmoid straight from PSUM.
                g = gp + ND
                ps2 = pspool.tile([P, NBLK * BT], f32, tag="mm", name="ps2")
                for x in range(nb):
                    nbl = bw // BT
                    if not staged:
                        wq = btiles[('gq', gp)]
                        for p in range(NP):
                            for b2 in range(nbl):
                                s2 = slice(x * bw + b2 * BT,
                                           x * bw + (b2 + 1) * BT)
                                nc.tensor.matmul(
                                    ps2[:, s2], wq[:, p, :, :],
                                    qp8[p][:, :, s2],
                                    start=(p == 0), stop=False,
                                    perf_mode=DR)
                    for p in range(NP):
                        for b2 in range(nbl):
                            s2 = slice(x * bw + b2 * BT,
                                       x * bw + (b2 + 1) * BT)
                            nc.tensor.matmul(
                                ps2[:, s2], wf[:, p, :, :], hh[p][:, :, s2],
                                start=(staged and p == 0),
                                stop=(p == NP - 1), perf_mode=DR)
                fg = gatepool.tile([P, T], f16, tag="gate", name="fg")
                tmp = scanpool.tile([P, T], f16, tag="scan", name="tmp")
                ob = opool.tile([P, T], f16, tag="ob", name="ob")
                for x in range(nb):
                    sl = slice(x * bw, (x + 1) * bw)
                    if staged:
                        nc.vector.tensor_add(fz[gp][:, sl], ps2[:, sl],
                                             fz[gp][:, sl])
                        nc.scalar.activation(fg[:, sl], fz[gp][:, sl], Sigm,
                                             bias=bT[:, 2 * ND + g:
                                                     2 * ND + g + 1])
                    else:
                        nc.scalar.activation(fg[:, sl], ps2[:, sl], Sigm,
                                             bias=bT[:, 2 * ND + g:
                                                     2 * ND + g + 1])
                    nc.vector.tensor_mul(tmp[:, sl], ig[:, sl],
                                         qT(gp)[:, sl])
                    nc.vector.tensor_mul(ob[:, sl], fg[:, sl],
                                         hh[gp // 2][:, gp % 2, sl])
                    nc.vector.tensor_add(ob[:, sl], ob[:, sl], tmp[:, sl])
                    dmae = nc.scalar if (last and x >= 2) else nc.gpsimd
                    dmae.dma_start(o[gp, :, sl], ob[:, sl])

    nc.finalize()
    return nc


_CACHED = {}
_last_result = None


def kernel(iQ, iV, W1, b1, W2, b2, Wg, bg):
    import sys
    if '/opt/trn_rl_repo' not in sys.path:
        sys.path.insert(0, '/opt/trn_rl_repo')
    import ml_dtypes
    from concourse.bass_utils import run_bass_kernel_spmd

    BF = ml_dtypes.bfloat16
    F16 = np.float16
    F8 = ml_dtypes.float8_e4m3

    iQ = np.asarray(iQ, np.float32)
    iV = np.asarray(iV, np.float32)
    W1 = np.asarray(W1, np.float32)
    b1 = np.asarray(b1, np.float32)
    W2 = np.asarray(W2, np.float32)
    b2 = np.asarray(b2, np.float32)
    Wg = np.asarray(Wg, np.float32)
    bg = np.asarray(bg, np.float32)

    if 'nc' not in _CACHED:
        _CACHED['nc'] = _build_program()
    nc = _CACHED['nc']

    # weight slabs, lhsT layouts (see _build_program dram shapes)
    def dr_slab(W, n_out):
        # [j, k, p, i, m] with K index (p*2+i)*128+k
        return np.ascontiguousarray(
            W.reshape(NP, 2, P, n_out, P).transpose(3, 2, 0, 1, 4)).astype(F8)

    w1s = dr_slab(W1, ND)
    w2s = dr_slab(W2, ND)
    wgtq = dr_slab(Wg[:D, :D], ND)
    wgtb = dr_slab(Wg[:D, D:], ND)
    wgb = dr_slab(Wg[D:], NG)
    bpk = np.ascontiguousarray(np.concatenate([
        b1.reshape(ND, P).T, b2.reshape(ND, P).T, bg.reshape(NG, P).T],
        axis=1))

    def t_orient(x, dt):  # [T, D] f32 -> [ND, P, T]
        return np.ascontiguousarray(x.T.reshape(ND, P, T)).astype(dt)

    def comp_ratio(n0):
        # f16 ratios r[t] ~ (n-1)/n chosen so the running product tracks
        # the ideal product to ~1 ulp (errors would otherwise compound
        # as a random walk across the 2048-step scan).
        r = np.empty(T, np.float32)
        perr = 1.0
        for i in range(T):
            n = n0 + i
            ideal = (n - 1) / n if n > 1 else 1.0
            c = np.float32(ideal / perr)
            cand = (np.nextafter(np.float16(c), np.float16(0)),
                    np.float16(c),
                    np.nextafter(np.float16(c), np.float16(2)))
            best = min((float(x) for x in cand),
                       key=lambda rv: abs(perr * rv / ideal - 1))
            r[i] = best
            perr = perr * best / ideal
        return r

    rats = {h: comp_ratio(h * T + 1) for h in (0, 1)}

    in_maps = []
    for core in range(8):
        b, half = core // 2, core % 2
        off = half * T
        ns = np.arange(off + 1, off + T + 1, dtype=np.float64)
        div = np.maximum(ns - 1, 1.0)
        rat = np.ascontiguousarray(
            np.broadcast_to(rats[half], (P, T))).astype(F16)
        if half == 1:
            car = np.ascontiguousarray(
                iV[b, :T].mean(axis=0, dtype=np.float64)
                .astype(np.float32).reshape(ND, P).T)
        else:
            car = np.zeros((P, ND), np.float32)
        m = {
            "qTd": t_orient(iQ[b, off:off + T], BF),
            "vwd": t_orient(
                (iV[b, off:off + T] / div[:, None]).astype(np.float32), F16),
            "carr": car,
            "ratio": rat,
            "w1s": w1s, "w2s": w2s, "wgb": wgb, "wgtb": wgtb,
            "wgtq": wgtq,
            "qp8d": np.ascontiguousarray(
                iQ[b, off:off + T].T.reshape(NP, 2, P, T)
                .transpose(0, 2, 1, 3)).astype(F8),
            "bpk": bpk,
        }
        in_maps.append(m)

    res = run_bass_kernel_spmd(nc, in_maps, core_ids=list(range(8)))
    global _last_result
    _last_result = res

    out = np.empty((B, S, D), np.float32)
    for core in range(8):
        b, half = core // 2, core % 2
        ot = np.asarray(res.results[core]["o"], dtype=np.float32)
        out[b, half * T:(half + 1) * T] = \
            ot.transpose(2, 0, 1).reshape(T, D)
    return out
